# revision 1
# baseline (speedup 1.0000x reference)
"""MoE + LoRA expert FFN kernel for 8 Trainium2 NeuronCores.

Strategy (expert-parallel, host dispatch/combine):
  - E=8 experts, one expert per core. The host groups tokens by expert
    (a token appears once per distinct selected expert; duplicate
    selections collapse with summed routing weight), pads each group to
    a uniform capacity C (multiple of 128), and ships per-core inputs:
        xT  [H, C]   tokens routed to this core's expert, transposed
        wg  [H, I]   gate_proj + 2*gate_A@gate_B   (LoRA folded)
        wu  [H, I]   up_proj   + 2*up_A@up_B
        wd  [I, H]   down_proj + 2*down_A@down_B
    and receives yT [H, C] fp32 = (silu(x@wg) * (x@wu)) @ wd, transposed.
  - Everything on device stays feature-major (features on partitions,
    tokens on the moving free dim) so no transposes are needed.
  - Matmuls run as float32r (replicated fp32): full fp32 storage, PE at
    ~bf16 speed for moving dims >= 256, ~1e-4 matmul relative error
    (measured) vs 4e-3 for bf16 inputs.
  - The host scales each token's expert output by its routing weight and
    scatters back into the [T, H] result.

LoRA folding is exact algebra: x@W + s*(x@A)@B == x@(W + s*A@B).
"""

import numpy as np
import ml_dtypes

E, H, I, R, TOPK = 8, 1024, 2816, 8, 2
SCALING = 2.0
NCORES = 8
KP = 128          # partition / contraction tile
NTOK = 512        # moving-dim (token) tile
MM_DTYPE = "float32r"     # "float32r" | "bfloat16"
BF16 = ml_dtypes.bfloat16

_cache = {}


def _setup_paths():
    import sys
    for p in ("/opt/trn_rl_repo", "/root/.axon_site"):
        if p not in sys.path:
            sys.path.insert(0, p)


def _split_multi_waits(nc):
    """The walrus in this container accepts at most 1 sem wait per
    instruction (2 on EventSemaphore); Tile emits more. Rewrite each block,
    moving excess waits onto preceding single-wait NoOps on the same
    engine (engines execute in order, so semantics are preserved)."""
    _setup_paths()
    from bass_rust import SyncInfo
    from concourse import mybir

    ctr = [0]
    for f in nc.m.functions:
        for bb in f.blocks:
            insts = bb.instructions
            new = []
            changed = False
            for inst in insts:
                si = inst.sync_info
                waits = list(si.on_wait or []) if si is not None else []
                cap = 2 if isinstance(inst, mybir.InstEventSemaphore) else 1
                if len(waits) > cap:
                    changed = True
                    for w in waits[:-cap]:
                        nop = mybir.InstNoOp(
                            name=f"SW-{ctr[0]}", ins=[], outs=[])
                        ctr[0] += 1
                        nop.engine = inst.engine
                        nop.sync_info = SyncInfo(on_wait=[w], on_update=[])
                        new.append(nop)
                    inst.sync_info = SyncInfo(
                        on_wait=waits[-cap:],
                        on_update=list(si.on_update or []))
                new.append(inst)
            if changed:
                bb.instructions = new


def _token_tiles(C):
    tiles = []
    t0 = 0
    while t0 < C:
        tw = min(NTOK, C - t0)
        tiles.append((t0, tw))
        t0 += tw
    return tiles


def _build(C):
    """Build the per-core Bass program for token capacity C (mult of 128)."""
    _setup_paths()
    import concourse.bass as bass
    import concourse.tile as tile
    from concourse import mybir

    f32 = mybir.dt.float32
    mm_r = MM_DTYPE == "float32r"
    # storage dtype for everything that feeds the PE: the BIR verifier
    # requires fp32r matmult inputs to be produced as fp32r, so the DRAM
    # params, weight/x/h tiles are typed float32r end-to-end (numpy side
    # still hands in plain fp32 bits).
    sdt = mybir.dt.float32r if mm_r else mybir.dt.bfloat16

    def mmv(ap):
        return ap

    KH = H // KP            # 8 contraction chunks over H
    KI = I // KP            # 22 chunks over I
    HH = H // KP            # 8 output row blocks

    nc = bass.Bass("TRN2", target_bir_lowering=False, debug=False,
                   num_devices=NCORES)
    xT = nc.declare_dram_parameter("xT", [H, C], sdt, isOutput=False)
    wg = nc.declare_dram_parameter("wg", [H, I], sdt, isOutput=False)
    wu = nc.declare_dram_parameter("wu", [H, I], sdt, isOutput=False)
    wd = nc.declare_dram_parameter("wd", [I, H], sdt, isOutput=False)
    yT = nc.declare_dram_parameter("yT", [H, C], f32, isOutput=True)

    ttiles = _token_tiles(C)

    # ramped weight column groups (in i-tiles): small first for fast start
    groups = [2, 2, 2]
    while sum(groups) < KI:
        groups.append(min(4, KI - sum(groups)))
    gstart = [sum(groups[:j]) for j in range(len(groups))]
    i2q = {}
    for qq, (g0, gn) in enumerate(zip(gstart, groups)):
        for i in range(g0, g0 + gn):
            i2q[i] = (qq, i - g0)

    with tile.TileContext(nc) as tc:
        with tc.tile_pool(name="hh", bufs=1) as hp, \
             tc.tile_pool(name="wpre", bufs=1) as wpre:
            h_t = [hp.tile([KP, C], sdt, tag=f"h{i}", name=f"h{i}")
                   for i in range(KI)]

            # a few wd tiles preloaded up-front so phase D starts without
            # waiting for phase B's SBUF region to free
            wd_pre = {}

            # ---- phase B: h = silu(x@wg) * (x@wu), feature-major [I, C]
            with tc.tile_pool(name="xp", bufs=1) as xp, \
                 tc.tile_pool(name="wst", bufs=32) as wst, \
                 tc.tile_pool(name="psB", bufs=4, space="PSUM") as psB, \
                 tc.tile_pool(name="actB", bufs=4) as actB:
                # x loads via SWDGE (gpsimd) — separate DMA queue rows, so
                # they don't contend with the two HWDGE rings carrying weights
                x_t = []
                for k in range(KH):
                    t = xp.tile([KP, C], sdt, tag=f"x{k}", name=f"x{k}")
                    nc.gpsimd.dma_start(
                        out=t, in_=xT[k * KP:(k + 1) * KP, :])
                    x_t.append(t)

                # ~4.5us of dummy matmuls so the PE HAM un-throttles to
                # 2.4 GHz while the first weight DMAs are in flight
                wsrc = actB.tile([KP, 256], mybir.dt.bfloat16,
                                 tag="wsrc", name="wsrc")
                nc.vector.memset(wsrc, 0.0)
                wdst = psB.tile([KP, 256], f32, tag="g", name="wdst")
                for w in range(38):
                    nc.tensor.matmul(wdst, mmv(wsrc[:, :128]), mmv(wsrc),
                                     start=(w == 0), stop=(w == 37))

                # streamed column-grouped weight loads (shared-tag pool).
                # wg rides the SP HWDGE ring; wu rides the ACT ring — the
                # first two groups up-front, later groups emitted just-in-
                # time inside the i-loop so ACT's silus are never queued
                # behind a long trigger backlog.
                wg_t, wu_t = {}, {}
                NG = len(groups)

                def load_w_group(q):
                    # allocation order must track consumption order — the
                    # shared-tag slot pool recycles FIFO
                    c0 = gstart[q] * KP
                    cw = groups[q] * KP
                    for k in range(KH):
                        t = wst.tile([KP, 4 * KP], sdt, tag="w",
                                     name=f"wg{k}_{q}")
                        nc.sync.dma_start(
                            out=t[:, :cw],
                            in_=wg[k * KP:(k + 1) * KP, c0:c0 + cw])
                        wg_t[(k, q)] = t
                        t = wst.tile([KP, 4 * KP], sdt, tag="w",
                                     name=f"wu{k}_{q}")
                        nc.scalar.dma_start(
                            out=t[:, :cw],
                            in_=wu[k * KP:(k + 1) * KP, c0:c0 + cw])
                        wu_t[(k, q)] = t

                for q in range(min(2, NG)):
                    load_w_group(q)
                for i in range(3):
                    t = wpre.tile([KP, H], sdt, tag=f"wpre{i}",
                                  name=f"wpre{i}")
                    nc.gpsimd.dma_start(
                        out=t, in_=wd[i * KP:(i + 1) * KP, :])
                    wd_pre[i] = t

                for i in range(KI):
                    q, r = i2q[i]
                    if r == 0 and q + 2 < NG:
                        load_w_group(q + 2)
                    isl = slice(r * KP, (r + 1) * KP)
                    for ti, (t0, tw) in enumerate(ttiles):
                        g_ps = psB.tile([KP, tw], f32, tag="g",
                                        name=f"g{i}_{t0}")
                        u_ps = psB.tile([KP, tw], f32, tag="u",
                                        name=f"u{i}_{t0}")
                        for k in range(KH):
                            nc.tensor.matmul(
                                g_ps, mmv(wg_t[(k, q)][:, isl]),
                                mmv(x_t[k][:, t0:t0 + tw]),
                                start=(k == 0), stop=(k == KH - 1))
                        for k in range(KH):
                            nc.tensor.matmul(
                                u_ps, mmv(wu_t[(k, q)][:, isl]),
                                mmv(x_t[k][:, t0:t0 + tw]),
                                start=(k == 0), stop=(k == KH - 1))
                        sg = actB.tile([KP, tw], f32, tag="sg",
                                       name=f"sg{i}_{t0}")
                        nc.scalar.activation(
                            sg, g_ps, mybir.ActivationFunctionType.Silu)
                        nc.vector.tensor_mul(
                            h_t[i][:, t0:t0 + tw], sg, u_ps)

            # ---- phase D: yT = h @ wd, output [H, C]
            # First token tile: i-outer, streaming wd tiles just-in-time
            # (the pool reuses the SBUF region phase B freed). Later token
            # tiles: hh-outer, reusing the resident wd tiles — each output
            # block finishes far apart, so the final copies/stores are
            # fully staggered and the kernel tail is short.
            with tc.tile_pool(name="wdp", bufs=1) as wdp, \
                 tc.tile_pool(name="yout", bufs=4) as yp, \
                 tc.tile_pool(name="psD", bufs=1, space="PSUM") as psD:
                wd_t = {}

                def emit_out(hh, t0, tw, y_ps):
                    yo = yp.tile([KP, tw], f32, tag="yo",
                                 name=f"yo{hh}_{t0}")
                    nc.vector.tensor_copy(yo, y_ps[hh])
                    nc.scalar.dma_start(
                        out=yT[hh * KP:(hh + 1) * KP, t0:t0 + tw],
                        in_=yo)

                for ti, (t0, tw) in enumerate(ttiles):
                    y_ps = [psD.tile([KP, tw], f32, tag=f"y{hh}",
                                     name=f"y{hh}_{t0}")
                            for hh in range(HH)]
                    if ti == 0:
                        for i in range(KI):
                            if i in wd_pre:
                                wd_t[i] = wd_pre[i]
                            else:
                                wdt = wdp.tile([KP, H], sdt, tag=f"wds{i}",
                                               name=f"wds{i}")
                                nc.gpsimd.dma_start(
                                    out=wdt, in_=wd[i * KP:(i + 1) * KP, :])
                                wd_t[i] = wdt
                            for hh in range(HH):
                                nc.tensor.matmul(
                                    y_ps[hh],
                                    mmv(wd_t[i][:, hh * KP:(hh + 1) * KP]),
                                    mmv(h_t[i][:, t0:t0 + tw]),
                                    start=(i == 0), stop=(i == KI - 1))
                        for hh in range(HH):
                            emit_out(hh, t0, tw, y_ps)
                    else:
                        for hh in range(HH):
                            for i in range(KI):
                                nc.tensor.matmul(
                                    y_ps[hh],
                                    mmv(wd_t[i][:, hh * KP:(hh + 1) * KP]),
                                    mmv(h_t[i][:, t0:t0 + tw]),
                                    start=(i == 0), stop=(i == KI - 1))
                            emit_out(hh, t0, tw, y_ps)
    _split_multi_waits(nc)
    return nc


CMAX = 1024   # per-run token capacity (bounded by SBUF for the h tiles)


def _prepare(inputs):
    """Host-side routing + weight folding. Returns (in_maps, idx, wts, C)."""
    hs = np.asarray(inputs["hidden_states"], dtype=np.float32)
    rw = np.asarray(inputs["routing_weights"], dtype=np.float32)
    se = np.asarray(inputs["selected_experts"]).astype(np.int64)
    T = hs.shape[0]

    combine = np.zeros((T, E), dtype=np.float32)
    for k in range(se.shape[1]):
        np.add.at(combine, (np.arange(T), se[:, k]), rw[:, k])

    idx = [np.nonzero(combine[:, e])[0] for e in range(E)]
    wts = [combine[idx[e], e] for e in range(E)]
    maxn = max((len(ix) for ix in idx), default=1)
    C = min(max(KP, -(-maxn // KP) * KP), CMAX)

    gp = np.asarray(inputs["gate_proj"], dtype=np.float32)
    up = np.asarray(inputs["up_proj"], dtype=np.float32)
    dp = np.asarray(inputs["down_proj"], dtype=np.float32)
    gA = np.asarray(inputs["gate_A"], dtype=np.float32)
    gB = np.asarray(inputs["gate_B"], dtype=np.float32)
    uA = np.asarray(inputs["up_A"], dtype=np.float32)
    uB = np.asarray(inputs["up_B"], dtype=np.float32)
    dA = np.asarray(inputs["down_A"], dtype=np.float32)
    dB = np.asarray(inputs["down_B"], dtype=np.float32)

    npdt = np.float32 if MM_DTYPE == "float32r" else BF16
    wmaps = []
    for e in range(E):
        wge = (gp[e] + SCALING * (gA[e] @ gB[e])).astype(npdt)
        wue = (up[e] + SCALING * (uA[e] @ uB[e])).astype(npdt)
        wde = (dp[e] + SCALING * (dA[e] @ dB[e])).astype(npdt)
        wmaps.append({"wg": wge, "wu": wue, "wd": wde})
    return hs, wmaps, idx, wts, C, npdt


def kernel(**inputs):
    _setup_paths()
    from concourse.bass_utils import run_bass_kernel_spmd

    hs, wmaps, idx, wts, C, npdt = _prepare(inputs)

    nc = _cache.get(C)
    if nc is None:
        nc = _build(C)
        _cache[C] = nc

    T = hs.shape[0]
    out = np.zeros((T, H), dtype=np.float32)
    maxn = max((len(ix) for ix in idx), default=1)
    nruns = max(1, -(-maxn // C))
    for r in range(nruns):
        in_maps = []
        for e in range(E):
            sub = idx[e][r * C:(r + 1) * C]
            xTe = np.zeros((H, C), dtype=npdt)
            if len(sub):
                xTe[:, :len(sub)] = hs[sub].T.astype(npdt)
            in_maps.append({"xT": xTe, **wmaps[e]})
        try:
            res = run_bass_kernel_spmd(
                nc, in_maps, core_ids=list(range(NCORES)))
        except Exception:
            import time
            time.sleep(2.0)
            res = run_bass_kernel_spmd(
                nc, in_maps, core_ids=list(range(NCORES)))

        # expose for external profiling harnesses (test.py)
        kernel._last = {"nc": nc, "in_maps": in_maps, "results": res}

        for e in range(E):
            sub = idx[e][r * C:(r + 1) * C]
            if not len(sub):
                continue
            w = wts[e][r * C:(r + 1) * C]
            yTe = res.results[e]["yT"]          # [H, C] fp32
            out[sub] += w[:, None] * yTe[:, :len(sub)].T
    return out



# revision 2
# speedup vs baseline: 1.0950x; 1.0950x over previous
"""MoE + LoRA expert FFN kernel for 8 Trainium2 NeuronCores.

Strategy (expert-parallel, host dispatch/combine):
  - E=8 experts, one expert per core. The host groups tokens by expert
    (a token appears once per distinct selected expert; duplicate
    selections collapse with summed routing weight), pads each group to
    a uniform capacity C (multiple of 128), and ships per-core inputs:
        xT  [H, C]   tokens routed to this core's expert, transposed
        wg  [H, I]   gate_proj + 2*gate_A@gate_B   (LoRA folded)
        wu  [H, I]   up_proj   + 2*up_A@up_B
        wd  [I, H]   down_proj + 2*down_A@down_B
    and receives yT [H, C] fp32 = (silu(x@wg) * (x@wu)) @ wd, transposed.
  - Everything on device stays feature-major (features on partitions,
    tokens on the moving free dim) so no transposes are needed.
  - Matmuls run as float32r (replicated fp32): full fp32 storage, PE at
    ~bf16 speed for moving dims >= 256, ~1e-4 matmul relative error
    (measured) vs 4e-3 for bf16 inputs.
  - The host scales each token's expert output by its routing weight and
    scatters back into the [T, H] result.

LoRA folding is exact algebra: x@W + s*(x@A)@B == x@(W + s*A@B).
"""

import numpy as np
import ml_dtypes

E, H, I, R, TOPK = 8, 1024, 2816, 8, 2
SCALING = 2.0
NCORES = 8
KP = 128          # partition / contraction tile
NTOK = 512        # moving-dim (token) tile
MM_DTYPE = "bfloat16"     # "float32r" | "bfloat16"
BF16 = ml_dtypes.bfloat16

_cache = {}


def _setup_paths():
    import sys
    for p in ("/opt/trn_rl_repo", "/root/.axon_site"):
        if p not in sys.path:
            sys.path.insert(0, p)


def _split_multi_waits(nc):
    """The walrus in this container accepts at most 1 sem wait per
    instruction (2 on EventSemaphore); Tile emits more. Rewrite each block,
    moving excess waits onto preceding single-wait NoOps on the same
    engine (engines execute in order, so semantics are preserved)."""
    _setup_paths()
    from bass_rust import SyncInfo
    from concourse import mybir

    ctr = [0]
    for f in nc.m.functions:
        for bb in f.blocks:
            insts = bb.instructions
            new = []
            changed = False
            for inst in insts:
                si = inst.sync_info
                waits = list(si.on_wait or []) if si is not None else []
                cap = 2 if isinstance(inst, mybir.InstEventSemaphore) else 1
                if len(waits) > cap:
                    changed = True
                    for w in waits[:-cap]:
                        nop = mybir.InstNoOp(
                            name=f"SW-{ctr[0]}", ins=[], outs=[])
                        ctr[0] += 1
                        nop.engine = inst.engine
                        nop.sync_info = SyncInfo(on_wait=[w], on_update=[])
                        new.append(nop)
                    inst.sync_info = SyncInfo(
                        on_wait=waits[-cap:],
                        on_update=list(si.on_update or []))
                new.append(inst)
            if changed:
                bb.instructions = new


def _token_tiles(C):
    tiles = []
    t0 = 0
    while t0 < C:
        tw = min(NTOK, C - t0)
        tiles.append((t0, tw))
        t0 += tw
    return tiles


def _build(C):
    """Build the per-core Bass program for token capacity C (mult of 128)."""
    _setup_paths()
    import concourse.bass as bass
    import concourse.tile as tile
    from concourse import mybir

    f32 = mybir.dt.float32
    mm_r = MM_DTYPE == "float32r"
    # storage dtype for everything that feeds the PE: the BIR verifier
    # requires fp32r matmult inputs to be produced as fp32r, so the DRAM
    # params, weight/x/h tiles are typed float32r end-to-end (numpy side
    # still hands in plain fp32 bits).
    sdt = mybir.dt.float32r if mm_r else mybir.dt.bfloat16

    def mmv(ap):
        return ap

    KH = H // KP            # 8 contraction chunks over H
    KI = I // KP            # 22 chunks over I
    HH = H // KP            # 8 output row blocks

    nc = bass.Bass("TRN2", target_bir_lowering=False, debug=False,
                   num_devices=NCORES)
    xT = nc.declare_dram_parameter("xT", [H, C], sdt, isOutput=False)
    wg = nc.declare_dram_parameter("wg", [H, I], sdt, isOutput=False)
    wu = nc.declare_dram_parameter("wu", [H, I], sdt, isOutput=False)
    wd = nc.declare_dram_parameter("wd", [I, H], sdt, isOutput=False)
    yT = nc.declare_dram_parameter("yT", [H, C], f32, isOutput=True)

    ttiles = _token_tiles(C)

    # ramped weight column groups (in i-tiles): small first for fast start
    groups = [2, 2, 2]
    while sum(groups) < KI:
        groups.append(min(4, KI - sum(groups)))
    gstart = [sum(groups[:j]) for j in range(len(groups))]
    i2q = {}
    for qq, (g0, gn) in enumerate(zip(gstart, groups)):
        for i in range(g0, g0 + gn):
            i2q[i] = (qq, i - g0)

    with tile.TileContext(nc) as tc:
        with tc.tile_pool(name="hh", bufs=1) as hp, \
             tc.tile_pool(name="wpre", bufs=1) as wpre:
            h_t = [hp.tile([KP, C], sdt, tag=f"h{i}", name=f"h{i}")
                   for i in range(KI)]

            # a few wd tiles preloaded up-front so phase D starts without
            # waiting for phase B's SBUF region to free
            wd_pre = {}

            # ---- phase B: h = silu(x@wg) * (x@wu), feature-major [I, C]
            with tc.tile_pool(name="xp", bufs=1) as xp, \
                 tc.tile_pool(name="wst", bufs=32) as wst, \
                 tc.tile_pool(name="psB", bufs=4, space="PSUM") as psB, \
                 tc.tile_pool(name="actB", bufs=4) as actB:
                # x loads via SWDGE (gpsimd) — separate DMA queue rows, so
                # they don't contend with the two HWDGE rings carrying weights
                x_t = []
                for k in range(KH):
                    t = xp.tile([KP, C], sdt, tag=f"x{k}", name=f"x{k}")
                    nc.gpsimd.dma_start(
                        out=t, in_=xT[k * KP:(k + 1) * KP, :])
                    x_t.append(t)

                # ~4.5us of dummy matmuls so the PE HAM un-throttles to
                # 2.4 GHz while the first weight DMAs are in flight
                wsrc = actB.tile([KP, 256], mybir.dt.bfloat16,
                                 tag="wsrc", name="wsrc")
                nc.vector.memset(wsrc, 0.0)
                wdst = psB.tile([KP, 256], f32, tag="g", name="wdst")
                for w in range(38):
                    nc.tensor.matmul(wdst, mmv(wsrc[:, :128]), mmv(wsrc),
                                     start=(w == 0), stop=(w == 37))

                # streamed column-grouped weight loads (shared-tag pool).
                # wg rides the SP HWDGE ring; wu rides the ACT ring — the
                # first two groups up-front, later groups emitted just-in-
                # time inside the i-loop so ACT's silus are never queued
                # behind a long trigger backlog.
                wg_t, wu_t = {}, {}
                NG = len(groups)

                def load_w_group(q):
                    # allocation order must track consumption order — the
                    # shared-tag slot pool recycles FIFO
                    c0 = gstart[q] * KP
                    cw = groups[q] * KP
                    for k in range(KH):
                        t = wst.tile([KP, 4 * KP], sdt, tag="w",
                                     name=f"wg{k}_{q}")
                        nc.sync.dma_start(
                            out=t[:, :cw],
                            in_=wg[k * KP:(k + 1) * KP, c0:c0 + cw])
                        wg_t[(k, q)] = t
                        t = wst.tile([KP, 4 * KP], sdt, tag="w",
                                     name=f"wu{k}_{q}")
                        nc.scalar.dma_start(
                            out=t[:, :cw],
                            in_=wu[k * KP:(k + 1) * KP, c0:c0 + cw])
                        wu_t[(k, q)] = t

                for q in range(min(2, NG)):
                    load_w_group(q)
                for i in range(3):
                    t = wpre.tile([KP, H], sdt, tag=f"wpre{i}",
                                  name=f"wpre{i}")
                    nc.gpsimd.dma_start(
                        out=t, in_=wd[i * KP:(i + 1) * KP, :])
                    wd_pre[i] = t

                for i in range(KI):
                    q, r = i2q[i]
                    if r == 0 and q + 2 < NG:
                        load_w_group(q + 2)
                    isl = slice(r * KP, (r + 1) * KP)
                    for ti, (t0, tw) in enumerate(ttiles):
                        g_ps = psB.tile([KP, tw], f32, tag="g",
                                        name=f"g{i}_{t0}")
                        u_ps = psB.tile([KP, tw], f32, tag="u",
                                        name=f"u{i}_{t0}")
                        for k in range(KH):
                            nc.tensor.matmul(
                                g_ps, mmv(wg_t[(k, q)][:, isl]),
                                mmv(x_t[k][:, t0:t0 + tw]),
                                start=(k == 0), stop=(k == KH - 1))
                        for k in range(KH):
                            nc.tensor.matmul(
                                u_ps, mmv(wu_t[(k, q)][:, isl]),
                                mmv(x_t[k][:, t0:t0 + tw]),
                                start=(k == 0), stop=(k == KH - 1))
                        sg = actB.tile([KP, tw], f32, tag="sg",
                                       name=f"sg{i}_{t0}")
                        nc.scalar.activation(
                            sg, g_ps, mybir.ActivationFunctionType.Silu)
                        nc.vector.tensor_mul(
                            h_t[i][:, t0:t0 + tw], sg, u_ps)

            # ---- phase D: yT = h @ wd, output [H, C]
            # First token tile: i-outer, streaming wd tiles just-in-time
            # (the pool reuses the SBUF region phase B freed). Later token
            # tiles: hh-outer, reusing the resident wd tiles — each output
            # block finishes far apart, so the final copies/stores are
            # fully staggered and the kernel tail is short.
            with tc.tile_pool(name="wdp", bufs=1) as wdp, \
                 tc.tile_pool(name="yout", bufs=4) as yp, \
                 tc.tile_pool(name="psD", bufs=1, space="PSUM") as psD:
                wd_t = {}

                def emit_out(hh, t0, tw, y_ps):
                    yo = yp.tile([KP, tw], f32, tag="yo",
                                 name=f"yo{hh}_{t0}")
                    nc.vector.tensor_copy(yo, y_ps[hh])
                    nc.scalar.dma_start(
                        out=yT[hh * KP:(hh + 1) * KP, t0:t0 + tw],
                        in_=yo)

                for ti, (t0, tw) in enumerate(ttiles):
                    y_ps = [psD.tile([KP, tw], f32, tag=f"y{hh}",
                                     name=f"y{hh}_{t0}")
                            for hh in range(HH)]
                    if ti == 0:
                        for i in range(KI):
                            if i in wd_pre:
                                wd_t[i] = wd_pre[i]
                            else:
                                wdt = wdp.tile([KP, H], sdt, tag=f"wds{i}",
                                               name=f"wds{i}")
                                nc.gpsimd.dma_start(
                                    out=wdt, in_=wd[i * KP:(i + 1) * KP, :])
                                wd_t[i] = wdt
                            for hh in range(HH):
                                nc.tensor.matmul(
                                    y_ps[hh],
                                    mmv(wd_t[i][:, hh * KP:(hh + 1) * KP]),
                                    mmv(h_t[i][:, t0:t0 + tw]),
                                    start=(i == 0), stop=(i == KI - 1))
                        for hh in range(HH):
                            emit_out(hh, t0, tw, y_ps)
                    else:
                        for hh in range(HH):
                            for i in range(KI):
                                nc.tensor.matmul(
                                    y_ps[hh],
                                    mmv(wd_t[i][:, hh * KP:(hh + 1) * KP]),
                                    mmv(h_t[i][:, t0:t0 + tw]),
                                    start=(i == 0), stop=(i == KI - 1))
                            emit_out(hh, t0, tw, y_ps)
    _split_multi_waits(nc)
    return nc


CMAX = 1024   # per-run token capacity (bounded by SBUF for the h tiles)


def _prepare(inputs):
    """Host-side routing + weight folding. Returns (in_maps, idx, wts, C)."""
    hs = np.asarray(inputs["hidden_states"], dtype=np.float32)
    rw = np.asarray(inputs["routing_weights"], dtype=np.float32)
    se = np.asarray(inputs["selected_experts"]).astype(np.int64)
    T = hs.shape[0]

    combine = np.zeros((T, E), dtype=np.float32)
    for k in range(se.shape[1]):
        np.add.at(combine, (np.arange(T), se[:, k]), rw[:, k])

    idx = [np.nonzero(combine[:, e])[0] for e in range(E)]
    wts = [combine[idx[e], e] for e in range(E)]
    maxn = max((len(ix) for ix in idx), default=1)
    C = min(max(KP, -(-maxn // KP) * KP), CMAX)

    gp = np.asarray(inputs["gate_proj"], dtype=np.float32)
    up = np.asarray(inputs["up_proj"], dtype=np.float32)
    dp = np.asarray(inputs["down_proj"], dtype=np.float32)
    gA = np.asarray(inputs["gate_A"], dtype=np.float32)
    gB = np.asarray(inputs["gate_B"], dtype=np.float32)
    uA = np.asarray(inputs["up_A"], dtype=np.float32)
    uB = np.asarray(inputs["up_B"], dtype=np.float32)
    dA = np.asarray(inputs["down_A"], dtype=np.float32)
    dB = np.asarray(inputs["down_B"], dtype=np.float32)

    npdt = np.float32 if MM_DTYPE == "float32r" else BF16
    wmaps = []
    for e in range(E):
        wge = (gp[e] + SCALING * (gA[e] @ gB[e])).astype(npdt)
        wue = (up[e] + SCALING * (uA[e] @ uB[e])).astype(npdt)
        wde = (dp[e] + SCALING * (dA[e] @ dB[e])).astype(npdt)
        wmaps.append({"wg": wge, "wu": wue, "wd": wde})
    return hs, wmaps, idx, wts, C, npdt


def kernel(**inputs):
    _setup_paths()
    from concourse.bass_utils import run_bass_kernel_spmd

    hs, wmaps, idx, wts, C, npdt = _prepare(inputs)

    nc = _cache.get(C)
    if nc is None:
        nc = _build(C)
        _cache[C] = nc

    T = hs.shape[0]
    out = np.zeros((T, H), dtype=np.float32)
    maxn = max((len(ix) for ix in idx), default=1)
    nruns = max(1, -(-maxn // C))
    for r in range(nruns):
        in_maps = []
        for e in range(E):
            sub = idx[e][r * C:(r + 1) * C]
            xTe = np.zeros((H, C), dtype=npdt)
            if len(sub):
                xTe[:, :len(sub)] = hs[sub].T.astype(npdt)
            in_maps.append({"xT": xTe, **wmaps[e]})
        try:
            res = run_bass_kernel_spmd(
                nc, in_maps, core_ids=list(range(NCORES)))
        except Exception:
            import time
            time.sleep(2.0)
            res = run_bass_kernel_spmd(
                nc, in_maps, core_ids=list(range(NCORES)))

        # expose for external profiling harnesses (test.py)
        kernel._last = {"nc": nc, "in_maps": in_maps, "results": res}

        for e in range(E):
            sub = idx[e][r * C:(r + 1) * C]
            if not len(sub):
                continue
            w = wts[e][r * C:(r + 1) * C]
            yTe = res.results[e]["yT"]          # [H, C] fp32
            out[sub] += w[:, None] * yTe[:, :len(sub)].T
    return out

